# revision 22
# baseline (speedup 1.0000x reference)
"""Bass/Tile TRN2 kernel for nn_BertSelfAttention2 (B=2, S=2048, D=1024, H=16).

Sharding: 8 cores = 2 (batch) x 4 (head groups of 4 heads). Each core
computes Q/K projections for its 4 heads (as 2 packed pairs), the modified
attention (kt = softplus(k), v = q + k, mask on the query axis), and writes
its [S, 256] slice of the output.

Layout trick: everything is computed in "T" orientation (scoresT[k, q]) so
no large on-device transposes are needed. The query-axis mask is applied by
zeroing masked query columns of Q; softmax of an all-zero score column then
reproduces the reference's uniform-probability behaviour for masked queries
exactly. The softmax denominator comes from a ones-column appended to V.
"""
import sys

if "/opt/trn_rl_repo" not in sys.path:
    sys.path.insert(0, "/opt/trn_rl_repo")

import numpy as np

B, S, D = 2, 2048, 1024
H = 16
HD = 64
NCORES = 8
HPC = H // (NCORES // B)     # heads per core = 4
NG = HPC // 2                # head-pair groups per core = 2
SC = 4                       # 512-wide query chunks
KC = S // 128                # 16 key chunks
SUPER = 2                    # key chunks per exp supertile

_CACHE = {}


def _build():
    import concourse.tile as tile
    from concourse import bacc, mybir
    from concourse.masks import make_identity

    F32 = mybir.dt.float32
    F32R = mybir.dt.float32r
    AF = mybir.ActivationFunctionType

    nc = bacc.Bacc(None, target_bir_lowering=False, debug=False)

    xt = nc.declare_dram_parameter("xt", [D, S], F32R, isOutput=False)
    wq = nc.declare_dram_parameter("wq", [D, 2 * 128], F32R, isOutput=False)
    wk = nc.declare_dram_parameter("wk", [D, 2 * 128], F32R, isOutput=False)
    bq = nc.declare_dram_parameter("bq", [2 * 128], F32, isOutput=False)
    bk = nc.declare_dram_parameter("bk", [2 * 128], F32, isOutput=False)
    maskb = nc.declare_dram_parameter("maskb", [1, S], F32, isOutput=False)
    out = nc.declare_dram_parameter("out", [S, 2 * 128], F32, isOutput=True)

    with tile.TileContext(nc) as tc:
        with tc.tile_pool(name="consts", bufs=1) as consts, \
             tc.tile_pool(name="big", bufs=1) as big, \
             tc.tile_pool(name="tmp", bufs=2) as tmp, \
             tc.tile_pool(name="expp", bufs=2) as expp, \
             tc.tile_pool(name="ep", bufs=2) as ep, \
             tc.tile_pool(name="ps_s", bufs=1, space="PSUM") as ps_s, \
             tc.tile_pool(name="ps_c", bufs=1, space="PSUM") as ps_c, \
             tc.tile_pool(name="ps_m", bufs=2, space="PSUM") as ps_m:

            ident = consts.tile([128, 128], F32)
            make_identity(nc, ident)

            # group-0 weights + small consts first, then X^T column-major so
            # the first projection chunk is ready ASAP.
            wq_t = [[consts.tile([128, 128], F32R, tag=f"wq{g}_{dchunk}",
                                 name=f"wq{g}_{dchunk}")
                     for dchunk in range(8)] for g in range(NG)]
            wk_t = [[consts.tile([128, 128], F32R, tag=f"wk{g}_{dchunk}",
                                 name=f"wk{g}_{dchunk}")
                     for dchunk in range(8)] for g in range(NG)]
            for dc in range(8):
                nc.sync.dma_start(
                    out=wq_t[0][dc], in_=wq[dc * 128:(dc + 1) * 128, 0:128])
                nc.sync.dma_start(
                    out=wk_t[0][dc], in_=wk[dc * 128:(dc + 1) * 128, 0:128])

            bq_t, bk_t = [], []
            for g in range(NG):
                bqt = consts.tile([128, 1], F32, tag=f"bq{g}", name=f"bq{g}")
                nc.sync.dma_start(
                    out=bqt,
                    in_=bq[g * 128:(g + 1) * 128].rearrange("(p o) -> p o", o=1))
                bq_t.append(bqt)
                bkt = consts.tile([128, 1], F32, tag=f"bk{g}", name=f"bk{g}")
                nc.sync.dma_start(
                    out=bkt,
                    in_=bk[g * 128:(g + 1) * 128].rearrange("(p o) -> p o", o=1))
                bk_t.append(bkt)

            # X^T as 32 [128, 512] tiles, loaded s-chunk-major; sc=0 first
            # (with the mask chunk it needs) so the first projection can
            # start as early as possible.
            xt_t = [[big.tile([128, 512], F32R, tag=f"xt{dchunk}_{scc}",
                              name=f"xt{dchunk}_{scc}")
                     for scc in range(SC)] for dchunk in range(8)]
            mask_t = [consts.tile([128, 512], F32, tag=f"mask{scc}",
                                  name=f"mask{scc}") for scc in range(SC)]
            mask_row = consts.tile([1, S], F32)
            nc.sync.dma_start(out=mask_row, in_=maskb[:, :])
            for scc in range(SC):
                nc.gpsimd.partition_broadcast(
                    mask_t[scc], mask_row[0:1, scc * 512:(scc + 1) * 512])
                for dc in range(8):
                    nc.sync.dma_start(
                        out=xt_t[dc][scc],
                        in_=xt[dc * 128:(dc + 1) * 128,
                               scc * 512:(scc + 1) * 512])

            for dc in range(8):
                nc.sync.dma_start(
                    out=wq_t[1][dc], in_=wq[dc * 128:(dc + 1) * 128, 128:256])
                nc.sync.dma_start(
                    out=wk_t[1][dc], in_=wk[dc * 128:(dc + 1) * 128, 128:256])

            # persistent activations, split into per-chunk tiles so the
            # scheduler sees fine-grained dependencies.
            # qt is stored twice with the other head's rows zeroed so the
            # scores matmuls run with a full K=128 contraction (reduced-K
            # fp32r matmuls do not register as PE activity for HAM and the
            # clock throttles to 1.2GHz). kt is the shared stationary.
            qtp = [[[big.tile([128, 512], F32R, tag=f"qtp{g}_{hh}_{scc}",
                              name=f"qtp{g}_{hh}_{scc}") for scc in range(SC)]
                    for hh in range(2)] for g in range(NG)]
            kt = [[big.tile([128, 512], F32R, tag=f"kt{g}_{scc}",
                            name=f"kt{g}_{scc}") for scc in range(SC)]
                  for g in range(NG)]
            vp = [[big.tile([128, 65], F32R, tag=f"vp{h}_{kc}",
                            name=f"vp{h}_{kc}") for kc in range(KC)]
                  for h in range(HPC)]

            def emit_vtrans(g, sc, vts):
                for hh in range(2):
                    h = g * 2 + hh
                    hsl = slice(hh * 64, (hh + 1) * 64)
                    for jj in range(4):
                        j = sc * 4 + jj
                        pv = ps_m.tile([128, 65], F32, tag="ep",
                                       name=f"pv{g}_{hh}_{j}")
                        nc.tensor.transpose(pv[:, 0:64],
                                            vts[hsl, jj * 128:(jj + 1) * 128],
                                            ident[hsl, hsl])
                        nc.vector.memset(pv[:, 64:65], 1.0)
                        nc.vector.tensor_copy(vp[h][j], pv)

            def proj_group(g, scs=None, vts_hist=None):
                if vts_hist is None:
                    vts_hist = []
                for sc in (range(SC) if scs is None else scs):
                    ssl = slice(sc * 512, (sc + 1) * 512)
                    pq = ps_m.tile([128, 512], F32, tag="ep", name=f"pq{g}_{sc}")
                    for dc in range(8):
                        nc.tensor.matmul(pq[:, 0:512], wq_t[g][dc],
                                         xt_t[dc][sc],
                                         start=(dc == 0), stop=(dc == 7))
                    pk = ps_m.tile([128, 512], F32, tag="ep", name=f"pk{g}_{sc}")
                    for dc in range(8):
                        nc.tensor.matmul(pk[:, 0:512], wk_t[g][dc],
                                         xt_t[dc][sc],
                                         start=(dc == 0), stop=(dc == 7))
                    tq = tmp.tile([128, 512], F32, tag="tq", name=f"tq{g}_{sc}")
                    nc.vector.tensor_scalar_add(tq, pq[:, 0:512], bq_t[g])
                    tk = tmp.tile([128, 512], F32, tag="tk", name=f"tk{g}_{sc}")
                    nc.vector.tensor_scalar_add(tk, pk[:, 0:512], bk_t[g])
                    # v = q + k (raw)
                    vts = tmp.tile([128, 512], F32, tag="vts", name=f"vts{g}_{sc}")
                    nc.vector.tensor_add(vts, tq, tk)
                    # masked q for scores, split per head into the
                    # zero-padded stores
                    nc.vector.tensor_mul(qtp[g][0][sc][0:64, :], tq[0:64, :],
                                         mask_t[sc][0:64, :])
                    nc.vector.tensor_scalar_mul(qtp[g][0][sc][64:128, :],
                                                tq[64:128, :], 0.0)
                    nc.vector.tensor_mul(qtp[g][1][sc][64:128, :], tq[64:128, :],
                                         mask_t[sc][64:128, :])
                    nc.vector.tensor_scalar_mul(qtp[g][1][sc][0:64, :],
                                                tq[0:64, :], 0.0)
                    # kt = softplus(k) = ln(exp(k) + 1)
                    te = tmp.tile([128, 512], F32, tag="tq", name=f"te{g}_{sc}")
                    nc.scalar.activation(out=te, in_=tk, func=AF.Exp)
                    nc.scalar.activation(out=kt[g][sc], in_=te,
                                         func=AF.Ln, bias=1.0)
                    vts_hist.append(vts)
                    # V' transposes run one s-chunk behind so the PE is never
                    # gated on this chunk's DVE chain
                    if sc > 0:
                        emit_vtrans(g, sc - 1, vts_hist[sc - 1])
                    if sc == SC - 1:
                        emit_vtrans(g, sc, vts_hist[sc])
                return vts_hist

            def attn_group(g, qcs=None):
                vpA = vp[g * 2]
                vpB = vp[g * 2 + 1]
                ktg = kt[g]
                for qc in (range(SC) if qcs is None else qcs):
                    qsl = slice(qc * 512, (qc + 1) * 512)
                    cA = ps_c.tile([65, 512], F32, tag="cA", name=f"cA{g}_{qc}")
                    cB = ps_c.tile([65, 512], F32, tag="cB", name=f"cB{g}_{qc}")
                    for st in range(KC // SUPER):
                        sA = ps_s.tile([128, SUPER * 512], F32, tag="sA",
                                       name=f"sA{g}_{qc}_{st}")
                        sB = ps_s.tile([128, SUPER * 512], F32, tag="sB",
                                       name=f"sB{g}_{qc}_{st}")
                        for kk in range(SUPER):
                            kc = st * SUPER + kk
                            osl = slice(kk * 512, (kk + 1) * 512)
                            lhs = ktg[kc // 4][:, (kc % 4) * 128:
                                                  (kc % 4 + 1) * 128]
                            nc.tensor.matmul(sA[:, osl], lhs,
                                             qtp[g][0][qc],
                                             start=True, stop=True)
                            nc.tensor.matmul(sB[:, osl], lhs,
                                             qtp[g][1][qc],
                                             start=True, stop=True)
                        eA = expp.tile([128, SUPER * 512], F32R, tag="eA",
                                       name=f"eA{g}_{qc}_{st}")
                        nc.scalar.activation(out=eA, in_=sA, func=AF.Exp,
                                             scale=0.125)
                        eB = expp.tile([128, SUPER * 512], F32R, tag="eB",
                                       name=f"eB{g}_{qc}_{st}")
                        nc.scalar.activation(out=eB, in_=sB, func=AF.Exp,
                                             scale=0.125)
                        for kk in range(SUPER):
                            kc = st * SUPER + kk
                            osl = slice(kk * 512, (kk + 1) * 512)
                            nc.tensor.matmul(cA, vpA[kc], eA[:, osl],
                                             start=(kc == 0), stop=(kc == KC - 1))
                            nc.tensor.matmul(cB, vpB[kc], eB[:, osl],
                                             start=(kc == 0), stop=(kc == KC - 1))
                    # epilogue: transpose ctxT back, normalize, store
                    csA = ep.tile([65, 512], F32, tag="csA", name=f"csA{g}_{qc}")
                    nc.vector.tensor_copy(csA, cA)
                    csB = ep.tile([65, 512], F32, tag="csB", name=f"csB{g}_{qc}")
                    nc.vector.tensor_copy(csB, cB)
                    for j in range(4):
                        jsl = slice(j * 128, (j + 1) * 128)
                        ptA = ps_c.tile([128, 65], F32, tag="cA",
                                        name=f"ptA{g}_{qc}_{j}")
                        nc.tensor.transpose(ptA[:, :], csA[:, jsl],
                                            ident[0:65, 0:65])
                        ptB = ps_c.tile([128, 65], F32, tag="cB",
                                        name=f"ptB{g}_{qc}_{j}")
                        nc.tensor.transpose(ptB[:, :], csB[:, jsl],
                                            ident[0:65, 0:65])
                        rA = ep.tile([128, 1], F32, tag="rA", name=f"rA{g}_{qc}_{j}")
                        nc.vector.reciprocal(rA, ptA[:, 64:65])
                        rB = ep.tile([128, 1], F32, tag="rB", name=f"rB{g}_{qc}_{j}")
                        nc.vector.reciprocal(rB, ptB[:, 64:65])
                        cf = ep.tile([128, 128], F32, tag="cf", name=f"cf{g}_{qc}_{j}")
                        nc.vector.tensor_scalar_mul(cf[:, 0:64], ptA[:, 0:64], rA)
                        nc.vector.tensor_scalar_mul(cf[:, 64:128], ptB[:, 0:64], rB)
                        nc.sync.dma_start(
                            out=out[qc * 512 + j * 128: qc * 512 + (j + 1) * 128,
                                    g * 128:(g + 1) * 128],
                            in_=cf)

            # proj(0); then att(0) q-chunks interleaved with proj(1)
            # s-chunks (the scheduler hides proj(1) in att(0)'s ACT-bound
            # PE gaps); then att(1).
            proj_group(0)
            hist = None
            for i in range(SC):
                attn_group(0, qcs=[i])
                hist = proj_group(1, scs=[i], vts_hist=hist)
            attn_group(1)

    nc.finalize()
    return nc


def _get_nc():
    if "nc" not in _CACHE:
        _CACHE["nc"] = _build()
    return _CACHE["nc"]


def _shard_inputs(hidden_states, attention_mask, Wq, bq, Wk, bk):
    hs = np.asarray(hidden_states, dtype=np.float32)
    am = np.asarray(attention_mask)
    Wq = np.asarray(Wq, dtype=np.float32)
    Wk = np.asarray(Wk, dtype=np.float32)
    bq = np.asarray(bq, dtype=np.float32)
    bk = np.asarray(bk, dtype=np.float32)

    xts = [np.ascontiguousarray(hs[b].T) for b in range(B)]
    maskbs = [np.ascontiguousarray(am[b].astype(np.float32)[None, :])
              for b in range(B)]

    in_maps = []
    for c in range(NCORES):
        b = c // (NCORES // B)
        hg = c % (NCORES // B)
        cols = slice(hg * 2 * 128, (hg + 1) * 2 * 128)
        in_maps.append({
            "xt": xts[b],
            "wq": np.ascontiguousarray(Wq[:, cols]),
            "wk": np.ascontiguousarray(Wk[:, cols]),
            "bq": np.ascontiguousarray(bq[cols]),
            "bk": np.ascontiguousarray(bk[cols]),
            "maskb": maskbs[b],
        })
    return in_maps


def _gather(results):
    full = np.empty((B, S, D), dtype=np.float32)
    for c in range(NCORES):
        b = c // (NCORES // B)
        hg = c % (NCORES // B)
        cols = slice(hg * 2 * 128, (hg + 1) * 2 * 128)
        full[b, :, cols] = results[c]["out"]
    return full


def run_sharded(in_maps, **kw):
    from concourse.bass_utils import run_bass_kernel_spmd
    nc = _get_nc()
    return run_bass_kernel_spmd(nc, in_maps, list(range(NCORES)), **kw)


def kernel(hidden_states, attention_mask, Wq, bq, Wk, bk):
    in_maps = _shard_inputs(hidden_states, attention_mask, Wq, bq, Wk, bk)
    res = run_sharded(in_maps)
    return _gather(res.results)


# revision 23
# speedup vs baseline: 1.0030x; 1.0030x over previous
"""Bass/Tile TRN2 kernel for nn_BertSelfAttention2 (B=2, S=2048, D=1024, H=16).

Sharding: 8 cores = 2 (batch) x 4 (head groups of 4 heads). Each core
computes Q/K projections for its 4 heads (as 2 packed pairs), the modified
attention (kt = softplus(k), v = q + k, mask on the query axis), and writes
its [S, 256] slice of the output.

Layout trick: everything is computed in "T" orientation (scoresT[k, q]) so
no large on-device transposes are needed. The query-axis mask is applied by
zeroing masked query columns of Q; softmax of an all-zero score column then
reproduces the reference's uniform-probability behaviour for masked queries
exactly. The softmax denominator comes from a ones-column appended to V.
"""
import sys

if "/opt/trn_rl_repo" not in sys.path:
    sys.path.insert(0, "/opt/trn_rl_repo")

import numpy as np

B, S, D = 2, 2048, 1024
H = 16
HD = 64
NCORES = 8
HPC = H // (NCORES // B)     # heads per core = 4
NG = HPC // 2                # head-pair groups per core = 2
SC = 4                       # 512-wide query chunks
KC = S // 128                # 16 key chunks
SUPER = 2                    # key chunks per exp supertile

_CACHE = {}


def _build():
    import concourse.tile as tile
    from concourse import bacc, mybir
    from concourse.masks import make_identity

    F32 = mybir.dt.float32
    F32R = mybir.dt.float32r
    AF = mybir.ActivationFunctionType

    nc = bacc.Bacc(None, target_bir_lowering=False, debug=False)

    xt = nc.declare_dram_parameter("xt", [D, S], F32R, isOutput=False)
    wq = nc.declare_dram_parameter("wq", [D, 2 * 128], F32R, isOutput=False)
    wk = nc.declare_dram_parameter("wk", [D, 2 * 128], F32R, isOutput=False)
    bq = nc.declare_dram_parameter("bq", [2 * 128], F32, isOutput=False)
    bk = nc.declare_dram_parameter("bk", [2 * 128], F32, isOutput=False)
    maskb = nc.declare_dram_parameter("maskb", [1, S], F32, isOutput=False)
    out = nc.declare_dram_parameter("out", [S, 2 * 128], F32, isOutput=True)

    with tile.TileContext(nc) as tc:
        with tc.tile_pool(name="consts", bufs=1) as consts, \
             tc.tile_pool(name="big", bufs=1) as big, \
             tc.tile_pool(name="tmp", bufs=2) as tmp, \
             tc.tile_pool(name="expp", bufs=2) as expp, \
             tc.tile_pool(name="ep", bufs=2) as ep, \
             tc.tile_pool(name="ps_s", bufs=1, space="PSUM") as ps_s, \
             tc.tile_pool(name="ps_c", bufs=1, space="PSUM") as ps_c, \
             tc.tile_pool(name="ps_m", bufs=2, space="PSUM") as ps_m:

            ident = consts.tile([128, 128], F32)
            make_identity(nc, ident)

            # group-0 weights + small consts first, then X^T column-major so
            # the first projection chunk is ready ASAP.
            wq_t = [[consts.tile([128, 128], F32R, tag=f"wq{g}_{dchunk}",
                                 name=f"wq{g}_{dchunk}")
                     for dchunk in range(8)] for g in range(NG)]
            wk_t = [[consts.tile([128, 128], F32R, tag=f"wk{g}_{dchunk}",
                                 name=f"wk{g}_{dchunk}")
                     for dchunk in range(8)] for g in range(NG)]
            for dc in range(8):
                nc.sync.dma_start(
                    out=wq_t[0][dc], in_=wq[dc * 128:(dc + 1) * 128, 0:128])
                nc.sync.dma_start(
                    out=wk_t[0][dc], in_=wk[dc * 128:(dc + 1) * 128, 0:128])

            bq_t, bk_t = [], []
            for g in range(NG):
                bqt = consts.tile([128, 1], F32, tag=f"bq{g}", name=f"bq{g}")
                nc.sync.dma_start(
                    out=bqt,
                    in_=bq[g * 128:(g + 1) * 128].rearrange("(p o) -> p o", o=1))
                bq_t.append(bqt)
                bkt = consts.tile([128, 1], F32, tag=f"bk{g}", name=f"bk{g}")
                nc.sync.dma_start(
                    out=bkt,
                    in_=bk[g * 128:(g + 1) * 128].rearrange("(p o) -> p o", o=1))
                bk_t.append(bkt)

            # X^T as 32 [128, 512] tiles, loaded s-chunk-major; sc=0 first
            # (with the mask chunk it needs) so the first projection can
            # start as early as possible.
            xt_t = [[big.tile([128, 512], F32R, tag=f"xt{dchunk}_{scc}",
                              name=f"xt{dchunk}_{scc}")
                     for scc in range(SC)] for dchunk in range(8)]
            mask_t = [consts.tile([128, 512], F32, tag=f"mask{scc}",
                                  name=f"mask{scc}") for scc in range(SC)]
            mask_row = consts.tile([1, S], F32)
            nc.sync.dma_start(out=mask_row, in_=maskb[:, :])
            for scc in range(SC):
                nc.gpsimd.partition_broadcast(
                    mask_t[scc], mask_row[0:1, scc * 512:(scc + 1) * 512])
                for dc in range(8):
                    nc.sync.dma_start(
                        out=xt_t[dc][scc],
                        in_=xt[dc * 128:(dc + 1) * 128,
                               scc * 512:(scc + 1) * 512])

            for dc in range(8):
                nc.sync.dma_start(
                    out=wq_t[1][dc], in_=wq[dc * 128:(dc + 1) * 128, 128:256])
                nc.sync.dma_start(
                    out=wk_t[1][dc], in_=wk[dc * 128:(dc + 1) * 128, 128:256])

            # persistent activations, split into per-chunk tiles so the
            # scheduler sees fine-grained dependencies.
            # qt is stored twice with the other head's rows zeroed so the
            # scores matmuls run with a full K=128 contraction (reduced-K
            # fp32r matmuls do not register as PE activity for HAM and the
            # clock throttles to 1.2GHz). kt is the shared stationary.
            qtp = [[[big.tile([128, 512], F32R, tag=f"qtp{g}_{hh}_{scc}",
                              name=f"qtp{g}_{hh}_{scc}") for scc in range(SC)]
                    for hh in range(2)] for g in range(NG)]
            kt = [[big.tile([128, 512], F32R, tag=f"kt{g}_{scc}",
                            name=f"kt{g}_{scc}") for scc in range(SC)]
                  for g in range(NG)]
            vp = [[big.tile([128, 65], F32R, tag=f"vp{h}_{kc}",
                            name=f"vp{h}_{kc}") for kc in range(KC)]
                  for h in range(HPC)]

            def emit_vtrans(g, sc, vts):
                for hh in range(2):
                    h = g * 2 + hh
                    hsl = slice(hh * 64, (hh + 1) * 64)
                    for jj in range(4):
                        j = sc * 4 + jj
                        pv = ps_m.tile([128, 65], F32, tag="ep",
                                       name=f"pv{g}_{hh}_{j}")
                        nc.tensor.transpose(pv[:, 0:64],
                                            vts[hsl, jj * 128:(jj + 1) * 128],
                                            ident[hsl, hsl])
                        nc.vector.memset(pv[:, 64:65], 1.0)
                        nc.vector.tensor_copy(vp[h][j], pv)

            def proj_group(g, scs=None, vts_hist=None):
                if vts_hist is None:
                    vts_hist = []
                for sc in (range(SC) if scs is None else scs):
                    ssl = slice(sc * 512, (sc + 1) * 512)
                    pq = ps_m.tile([128, 512], F32, tag="ep", name=f"pq{g}_{sc}")
                    for dc in range(8):
                        nc.tensor.matmul(pq[:, 0:512], wq_t[g][dc],
                                         xt_t[dc][sc],
                                         start=(dc == 0), stop=(dc == 7))
                    pk = ps_m.tile([128, 512], F32, tag="ep", name=f"pk{g}_{sc}")
                    for dc in range(8):
                        nc.tensor.matmul(pk[:, 0:512], wk_t[g][dc],
                                         xt_t[dc][sc],
                                         start=(dc == 0), stop=(dc == 7))
                    tq = tmp.tile([128, 512], F32, tag="tq", name=f"tq{g}_{sc}")
                    nc.vector.tensor_scalar_add(tq, pq[:, 0:512], bq_t[g])
                    tk = tmp.tile([128, 512], F32, tag="tk", name=f"tk{g}_{sc}")
                    nc.vector.tensor_scalar_add(tk, pk[:, 0:512], bk_t[g])
                    # v = q + k (raw)
                    vts = tmp.tile([128, 512], F32, tag="vts", name=f"vts{g}_{sc}")
                    nc.vector.tensor_add(vts, tq, tk)
                    # masked q for scores, split per head into the
                    # zero-padded stores
                    nc.vector.tensor_mul(qtp[g][0][sc][0:64, :], tq[0:64, :],
                                         mask_t[sc][0:64, :])
                    nc.vector.tensor_scalar_mul(qtp[g][0][sc][64:128, :],
                                                tq[64:128, :], 0.0)
                    nc.vector.tensor_mul(qtp[g][1][sc][64:128, :], tq[64:128, :],
                                         mask_t[sc][64:128, :])
                    nc.vector.tensor_scalar_mul(qtp[g][1][sc][0:64, :],
                                                tq[0:64, :], 0.0)
                    # kt = softplus(k) = ln(exp(k) + 1)
                    te = tmp.tile([128, 512], F32, tag="tq", name=f"te{g}_{sc}")
                    nc.scalar.activation(out=te, in_=tk, func=AF.Exp)
                    nc.scalar.activation(out=kt[g][sc], in_=te,
                                         func=AF.Ln, bias=1.0)
                    vts_hist.append(vts)
                    # V' transposes run one s-chunk behind so the PE is never
                    # gated on this chunk's DVE chain
                    if sc > 0:
                        emit_vtrans(g, sc - 1, vts_hist[sc - 1])
                    if sc == SC - 1:
                        emit_vtrans(g, sc, vts_hist[sc])
                return vts_hist

            def attn_group(g, qcs=None):
                vpA = vp[g * 2]
                vpB = vp[g * 2 + 1]
                ktg = kt[g]
                for qc in (range(SC) if qcs is None else qcs):
                    qsl = slice(qc * 512, (qc + 1) * 512)
                    cA = ps_c.tile([65, 512], F32, tag="cA", name=f"cA{g}_{qc}")
                    cB = ps_c.tile([65, 512], F32, tag="cB", name=f"cB{g}_{qc}")
                    for st in range(KC // SUPER):
                        sA = ps_s.tile([128, SUPER * 512], F32, tag="sA",
                                       name=f"sA{g}_{qc}_{st}")
                        sB = ps_s.tile([128, SUPER * 512], F32, tag="sB",
                                       name=f"sB{g}_{qc}_{st}")
                        for kk in range(SUPER):
                            kc = st * SUPER + kk
                            osl = slice(kk * 512, (kk + 1) * 512)
                            lhs = ktg[kc // 4][:, (kc % 4) * 128:
                                                  (kc % 4 + 1) * 128]
                            nc.tensor.matmul(sA[:, osl], lhs,
                                             qtp[g][0][qc],
                                             start=True, stop=True)
                            nc.tensor.matmul(sB[:, osl], lhs,
                                             qtp[g][1][qc],
                                             start=True, stop=True)
                        eA = expp.tile([128, SUPER * 512], F32R, tag="eA",
                                       name=f"eA{g}_{qc}_{st}")
                        nc.scalar.activation(out=eA, in_=sA, func=AF.Exp,
                                             scale=0.125)
                        eB = expp.tile([128, SUPER * 512], F32R, tag="eB",
                                       name=f"eB{g}_{qc}_{st}")
                        nc.scalar.activation(out=eB, in_=sB, func=AF.Exp,
                                             scale=0.125)
                        for kk in range(SUPER):
                            kc = st * SUPER + kk
                            osl = slice(kk * 512, (kk + 1) * 512)
                            nc.tensor.matmul(cA, vpA[kc], eA[:, osl],
                                             start=(kc == 0), stop=(kc == KC - 1))
                            nc.tensor.matmul(cB, vpB[kc], eB[:, osl],
                                             start=(kc == 0), stop=(kc == KC - 1))
                    # epilogue: transpose ctxT back, normalize, store
                    csA = ep.tile([65, 512], F32, tag="csA", name=f"csA{g}_{qc}")
                    nc.vector.tensor_copy(csA, cA)
                    csB = ep.tile([65, 512], F32, tag="csB", name=f"csB{g}_{qc}")
                    nc.vector.tensor_copy(csB, cB)
                    for j in range(4):
                        jsl = slice(j * 128, (j + 1) * 128)
                        ptA = ps_c.tile([128, 65], F32, tag="cA",
                                        name=f"ptA{g}_{qc}_{j}")
                        nc.tensor.transpose(ptA[:, :], csA[:, jsl],
                                            ident[0:65, 0:65])
                        ptB = ps_c.tile([128, 65], F32, tag="cB",
                                        name=f"ptB{g}_{qc}_{j}")
                        nc.tensor.transpose(ptB[:, :], csB[:, jsl],
                                            ident[0:65, 0:65])
                        rA = ep.tile([128, 1], F32, tag="rA", name=f"rA{g}_{qc}_{j}")
                        nc.vector.reciprocal(rA, ptA[:, 64:65])
                        rB = ep.tile([128, 1], F32, tag="rB", name=f"rB{g}_{qc}_{j}")
                        nc.vector.reciprocal(rB, ptB[:, 64:65])
                        cf = ep.tile([128, 128], F32, tag="cf", name=f"cf{g}_{qc}_{j}")
                        nc.vector.tensor_scalar_mul(cf[:, 0:64], ptA[:, 0:64], rA)
                        nc.vector.tensor_scalar_mul(cf[:, 64:128], ptB[:, 0:64], rB)
                        nc.sync.dma_start(
                            out=out[qc * 512 + j * 128: qc * 512 + (j + 1) * 128,
                                    g * 128:(g + 1) * 128],
                            in_=cf)

            for g in range(NG):
                proj_group(g)
                attn_group(g)

    nc.finalize()
    return nc


def _get_nc():
    if "nc" not in _CACHE:
        _CACHE["nc"] = _build()
    return _CACHE["nc"]


def _shard_inputs(hidden_states, attention_mask, Wq, bq, Wk, bk):
    hs = np.asarray(hidden_states, dtype=np.float32)
    am = np.asarray(attention_mask)
    Wq = np.asarray(Wq, dtype=np.float32)
    Wk = np.asarray(Wk, dtype=np.float32)
    bq = np.asarray(bq, dtype=np.float32)
    bk = np.asarray(bk, dtype=np.float32)

    xts = [np.ascontiguousarray(hs[b].T) for b in range(B)]
    maskbs = [np.ascontiguousarray(am[b].astype(np.float32)[None, :])
              for b in range(B)]

    in_maps = []
    for c in range(NCORES):
        b = c // (NCORES // B)
        hg = c % (NCORES // B)
        cols = slice(hg * 2 * 128, (hg + 1) * 2 * 128)
        in_maps.append({
            "xt": xts[b],
            "wq": np.ascontiguousarray(Wq[:, cols]),
            "wk": np.ascontiguousarray(Wk[:, cols]),
            "bq": np.ascontiguousarray(bq[cols]),
            "bk": np.ascontiguousarray(bk[cols]),
            "maskb": maskbs[b],
        })
    return in_maps


def _gather(results):
    full = np.empty((B, S, D), dtype=np.float32)
    for c in range(NCORES):
        b = c // (NCORES // B)
        hg = c % (NCORES // B)
        cols = slice(hg * 2 * 128, (hg + 1) * 2 * 128)
        full[b, :, cols] = results[c]["out"]
    return full


def run_sharded(in_maps, **kw):
    from concourse.bass_utils import run_bass_kernel_spmd
    nc = _get_nc()
    return run_bass_kernel_spmd(nc, in_maps, list(range(NCORES)), **kw)


def kernel(hidden_states, attention_mask, Wq, bq, Wk, bk):
    in_maps = _shard_inputs(hidden_states, attention_mask, Wq, bq, Wk, bk)
    res = run_sharded(in_maps)
    return _gather(res.results)


# revision 24
# speedup vs baseline: 1.0096x; 1.0065x over previous
"""Bass/Tile TRN2 kernel for nn_BertSelfAttention2 (B=2, S=2048, D=1024, H=16).

Sharding: 8 cores = 2 (batch) x 4 (head groups of 4 heads). Each core
computes Q/K projections for its 4 heads (as 2 packed pairs), the modified
attention (kt = softplus(k), v = q + k, mask on the query axis), and writes
its [S, 256] slice of the output.

Layout trick: everything is computed in "T" orientation (scoresT[k, q]) so
no large on-device transposes are needed. The query-axis mask is applied by
zeroing masked query columns of Q; softmax of an all-zero score column then
reproduces the reference's uniform-probability behaviour for masked queries
exactly. The softmax denominator comes from a ones-column appended to V.
"""
import sys

if "/opt/trn_rl_repo" not in sys.path:
    sys.path.insert(0, "/opt/trn_rl_repo")

import numpy as np

B, S, D = 2, 2048, 1024
H = 16
HD = 64
NCORES = 8
HPC = H // (NCORES // B)     # heads per core = 4
NG = HPC // 2                # head-pair groups per core = 2
SC = 4                       # 512-wide query chunks
KC = S // 128                # 16 key chunks
SUPER = 2                    # key chunks per exp supertile

_CACHE = {}


def _build():
    import concourse.tile as tile
    from concourse import bacc, mybir
    from concourse.masks import make_identity

    F32 = mybir.dt.float32
    F32R = mybir.dt.float32r
    AF = mybir.ActivationFunctionType

    nc = bacc.Bacc(None, target_bir_lowering=False, debug=False)

    # all tiled operands are shipped pre-tiled so every SBUF tile load is
    # one contiguous DRAM read (strided row reads leave the DMA engines
    # descriptor-bound at ~1/3 utilization)
    xt = nc.declare_dram_parameter("xt", [SC * 8 * 128, 512], F32R, isOutput=False)
    wq = nc.declare_dram_parameter("wq", [NG * 8 * 128, 128], F32R, isOutput=False)
    wk = nc.declare_dram_parameter("wk", [NG * 8 * 128, 128], F32R, isOutput=False)
    bq = nc.declare_dram_parameter("bq", [2 * 128], F32, isOutput=False)
    bk = nc.declare_dram_parameter("bk", [2 * 128], F32, isOutput=False)
    maskb = nc.declare_dram_parameter("maskb", [1, S], F32, isOutput=False)
    out = nc.declare_dram_parameter("out", [NG * S, 128], F32, isOutput=True)

    with tile.TileContext(nc) as tc:
        with tc.tile_pool(name="consts", bufs=1) as consts, \
             tc.tile_pool(name="big", bufs=1) as big, \
             tc.tile_pool(name="tmp", bufs=2) as tmp, \
             tc.tile_pool(name="expp", bufs=2) as expp, \
             tc.tile_pool(name="ep", bufs=2) as ep, \
             tc.tile_pool(name="ps_s", bufs=1, space="PSUM") as ps_s, \
             tc.tile_pool(name="ps_c", bufs=1, space="PSUM") as ps_c, \
             tc.tile_pool(name="ps_m", bufs=2, space="PSUM") as ps_m:

            ident = consts.tile([128, 128], F32)
            make_identity(nc, ident)

            # group-0 weights + small consts first, then X^T column-major so
            # the first projection chunk is ready ASAP.
            wq_t = [[consts.tile([128, 128], F32R, tag=f"wq{g}_{dchunk}",
                                 name=f"wq{g}_{dchunk}")
                     for dchunk in range(8)] for g in range(NG)]
            wk_t = [[consts.tile([128, 128], F32R, tag=f"wk{g}_{dchunk}",
                                 name=f"wk{g}_{dchunk}")
                     for dchunk in range(8)] for g in range(NG)]
            for dc in range(8):
                nc.sync.dma_start(
                    out=wq_t[0][dc], in_=wq[dc * 128:(dc + 1) * 128, :])
                nc.sync.dma_start(
                    out=wk_t[0][dc], in_=wk[dc * 128:(dc + 1) * 128, :])

            bq_t, bk_t = [], []
            for g in range(NG):
                bqt = consts.tile([128, 1], F32, tag=f"bq{g}", name=f"bq{g}")
                nc.sync.dma_start(
                    out=bqt,
                    in_=bq[g * 128:(g + 1) * 128].rearrange("(p o) -> p o", o=1))
                bq_t.append(bqt)
                bkt = consts.tile([128, 1], F32, tag=f"bk{g}", name=f"bk{g}")
                nc.sync.dma_start(
                    out=bkt,
                    in_=bk[g * 128:(g + 1) * 128].rearrange("(p o) -> p o", o=1))
                bk_t.append(bkt)

            # X^T as 32 [128, 512] tiles, loaded s-chunk-major; sc=0 first
            # (with the mask chunk it needs) so the first projection can
            # start as early as possible.
            xt_t = [[big.tile([128, 512], F32R, tag=f"xt{dchunk}_{scc}",
                              name=f"xt{dchunk}_{scc}")
                     for scc in range(SC)] for dchunk in range(8)]
            mask_t = [consts.tile([128, 512], F32, tag=f"mask{scc}",
                                  name=f"mask{scc}") for scc in range(SC)]
            mask_row = consts.tile([1, S], F32)
            nc.sync.dma_start(out=mask_row, in_=maskb[:, :])
            for scc in range(SC):
                nc.gpsimd.partition_broadcast(
                    mask_t[scc], mask_row[0:1, scc * 512:(scc + 1) * 512])
                for dc in range(8):
                    base = (scc * 8 + dc) * 128
                    nc.sync.dma_start(out=xt_t[dc][scc],
                                      in_=xt[base:base + 128, :])

            for dc in range(8):
                base = (8 + dc) * 128
                nc.sync.dma_start(out=wq_t[1][dc], in_=wq[base:base + 128, :])
                nc.sync.dma_start(out=wk_t[1][dc], in_=wk[base:base + 128, :])

            # persistent activations, split into per-chunk tiles so the
            # scheduler sees fine-grained dependencies.
            # qt is stored twice with the other head's rows zeroed so the
            # scores matmuls run with a full K=128 contraction (reduced-K
            # fp32r matmuls do not register as PE activity for HAM and the
            # clock throttles to 1.2GHz). kt is the shared stationary.
            qtp = [[[big.tile([128, 512], F32R, tag=f"qtp{g}_{hh}_{scc}",
                              name=f"qtp{g}_{hh}_{scc}") for scc in range(SC)]
                    for hh in range(2)] for g in range(NG)]
            kt = [[big.tile([128, 512], F32R, tag=f"kt{g}_{scc}",
                            name=f"kt{g}_{scc}") for scc in range(SC)]
                  for g in range(NG)]
            vp = [[big.tile([128, 65], F32R, tag=f"vp{h}_{kc}",
                            name=f"vp{h}_{kc}") for kc in range(KC)]
                  for h in range(HPC)]

            def emit_vtrans(g, sc, vts):
                for hh in range(2):
                    h = g * 2 + hh
                    hsl = slice(hh * 64, (hh + 1) * 64)
                    for jj in range(4):
                        j = sc * 4 + jj
                        pv = ps_m.tile([128, 65], F32, tag="ep",
                                       name=f"pv{g}_{hh}_{j}")
                        nc.tensor.transpose(pv[:, 0:64],
                                            vts[hsl, jj * 128:(jj + 1) * 128],
                                            ident[hsl, hsl])
                        nc.vector.memset(pv[:, 64:65], 1.0)
                        nc.vector.tensor_copy(vp[h][j], pv)

            def proj_group(g, scs=None, vts_hist=None):
                if vts_hist is None:
                    vts_hist = []
                for sc in (range(SC) if scs is None else scs):
                    ssl = slice(sc * 512, (sc + 1) * 512)
                    pq = ps_m.tile([128, 512], F32, tag="ep", name=f"pq{g}_{sc}")
                    for dc in range(8):
                        nc.tensor.matmul(pq[:, 0:512], wq_t[g][dc],
                                         xt_t[dc][sc],
                                         start=(dc == 0), stop=(dc == 7))
                    pk = ps_m.tile([128, 512], F32, tag="ep", name=f"pk{g}_{sc}")
                    for dc in range(8):
                        nc.tensor.matmul(pk[:, 0:512], wk_t[g][dc],
                                         xt_t[dc][sc],
                                         start=(dc == 0), stop=(dc == 7))
                    tq = tmp.tile([128, 512], F32, tag="tq", name=f"tq{g}_{sc}")
                    nc.vector.tensor_scalar_add(tq, pq[:, 0:512], bq_t[g])
                    tk = tmp.tile([128, 512], F32, tag="tk", name=f"tk{g}_{sc}")
                    nc.vector.tensor_scalar_add(tk, pk[:, 0:512], bk_t[g])
                    # v = q + k (raw)
                    vts = tmp.tile([128, 512], F32, tag="vts", name=f"vts{g}_{sc}")
                    nc.vector.tensor_add(vts, tq, tk)
                    # masked q for scores, split per head into the
                    # zero-padded stores
                    nc.vector.tensor_mul(qtp[g][0][sc][0:64, :], tq[0:64, :],
                                         mask_t[sc][0:64, :])
                    nc.vector.tensor_scalar_mul(qtp[g][0][sc][64:128, :],
                                                tq[64:128, :], 0.0)
                    nc.vector.tensor_mul(qtp[g][1][sc][64:128, :], tq[64:128, :],
                                         mask_t[sc][64:128, :])
                    nc.vector.tensor_scalar_mul(qtp[g][1][sc][0:64, :],
                                                tq[0:64, :], 0.0)
                    # kt = softplus(k) = ln(exp(k) + 1)
                    te = tmp.tile([128, 512], F32, tag="tq", name=f"te{g}_{sc}")
                    nc.scalar.activation(out=te, in_=tk, func=AF.Exp)
                    nc.scalar.activation(out=kt[g][sc], in_=te,
                                         func=AF.Ln, bias=1.0)
                    vts_hist.append(vts)
                    # V' transposes run one s-chunk behind so the PE is never
                    # gated on this chunk's DVE chain
                    if sc > 0:
                        emit_vtrans(g, sc - 1, vts_hist[sc - 1])
                    if sc == SC - 1:
                        emit_vtrans(g, sc, vts_hist[sc])
                return vts_hist

            def attn_group(g, qcs=None):
                vpA = vp[g * 2]
                vpB = vp[g * 2 + 1]
                ktg = kt[g]
                for qc in (range(SC) if qcs is None else qcs):
                    qsl = slice(qc * 512, (qc + 1) * 512)
                    cA = ps_c.tile([65, 512], F32, tag="cA", name=f"cA{g}_{qc}")
                    cB = ps_c.tile([65, 512], F32, tag="cB", name=f"cB{g}_{qc}")
                    for st in range(KC // SUPER):
                        sA = ps_s.tile([128, SUPER * 512], F32, tag="sA",
                                       name=f"sA{g}_{qc}_{st}")
                        sB = ps_s.tile([128, SUPER * 512], F32, tag="sB",
                                       name=f"sB{g}_{qc}_{st}")
                        for kk in range(SUPER):
                            kc = st * SUPER + kk
                            osl = slice(kk * 512, (kk + 1) * 512)
                            lhs = ktg[kc // 4][:, (kc % 4) * 128:
                                                  (kc % 4 + 1) * 128]
                            nc.tensor.matmul(sA[:, osl], lhs,
                                             qtp[g][0][qc],
                                             start=True, stop=True)
                            nc.tensor.matmul(sB[:, osl], lhs,
                                             qtp[g][1][qc],
                                             start=True, stop=True)
                        eA = expp.tile([128, SUPER * 512], F32R, tag="eA",
                                       name=f"eA{g}_{qc}_{st}")
                        nc.scalar.activation(out=eA, in_=sA, func=AF.Exp,
                                             scale=0.125)
                        eB = expp.tile([128, SUPER * 512], F32R, tag="eB",
                                       name=f"eB{g}_{qc}_{st}")
                        nc.scalar.activation(out=eB, in_=sB, func=AF.Exp,
                                             scale=0.125)
                        for kk in range(SUPER):
                            kc = st * SUPER + kk
                            osl = slice(kk * 512, (kk + 1) * 512)
                            nc.tensor.matmul(cA, vpA[kc], eA[:, osl],
                                             start=(kc == 0), stop=(kc == KC - 1))
                            nc.tensor.matmul(cB, vpB[kc], eB[:, osl],
                                             start=(kc == 0), stop=(kc == KC - 1))
                    # epilogue: transpose ctxT back, normalize, store
                    csA = ep.tile([65, 512], F32, tag="csA", name=f"csA{g}_{qc}")
                    nc.vector.tensor_copy(csA, cA)
                    csB = ep.tile([65, 512], F32, tag="csB", name=f"csB{g}_{qc}")
                    nc.vector.tensor_copy(csB, cB)
                    for j in range(4):
                        jsl = slice(j * 128, (j + 1) * 128)
                        ptA = ps_c.tile([128, 65], F32, tag="cA",
                                        name=f"ptA{g}_{qc}_{j}")
                        nc.tensor.transpose(ptA[:, :], csA[:, jsl],
                                            ident[0:65, 0:65])
                        ptB = ps_c.tile([128, 65], F32, tag="cB",
                                        name=f"ptB{g}_{qc}_{j}")
                        nc.tensor.transpose(ptB[:, :], csB[:, jsl],
                                            ident[0:65, 0:65])
                        rA = ep.tile([128, 1], F32, tag="rA", name=f"rA{g}_{qc}_{j}")
                        nc.vector.reciprocal(rA, ptA[:, 64:65])
                        rB = ep.tile([128, 1], F32, tag="rB", name=f"rB{g}_{qc}_{j}")
                        nc.vector.reciprocal(rB, ptB[:, 64:65])
                        cf = ep.tile([128, 128], F32, tag="cf", name=f"cf{g}_{qc}_{j}")
                        nc.vector.tensor_scalar_mul(cf[:, 0:64], ptA[:, 0:64], rA)
                        nc.vector.tensor_scalar_mul(cf[:, 64:128], ptB[:, 0:64], rB)
                        row = g * S + qc * 512 + j * 128
                        nc.sync.dma_start(out=out[row:row + 128, :], in_=cf)

            for g in range(NG):
                proj_group(g)
                attn_group(g)

    nc.finalize()
    return nc


def _get_nc():
    if "nc" not in _CACHE:
        _CACHE["nc"] = _build()
    return _CACHE["nc"]


def _shard_inputs(hidden_states, attention_mask, Wq, bq, Wk, bk):
    hs = np.asarray(hidden_states, dtype=np.float32)
    am = np.asarray(attention_mask)
    Wq = np.asarray(Wq, dtype=np.float32)
    Wk = np.asarray(Wk, dtype=np.float32)
    bq = np.asarray(bq, dtype=np.float32)
    bk = np.asarray(bk, dtype=np.float32)

    xts = [np.ascontiguousarray(
        hs[b].T.reshape(8, 128, SC, 512).transpose(2, 0, 1, 3)
        .reshape(SC * 8 * 128, 512)) for b in range(B)]
    maskbs = [np.ascontiguousarray(am[b].astype(np.float32)[None, :])
              for b in range(B)]

    in_maps = []
    for c in range(NCORES):
        b = c // (NCORES // B)
        hg = c % (NCORES // B)
        cols = slice(hg * 2 * 128, (hg + 1) * 2 * 128)
        def _tile_w(W):
            return np.ascontiguousarray(
                W[:, cols].reshape(8, 128, NG, 128).transpose(2, 0, 1, 3)
                .reshape(NG * 8 * 128, 128))
        in_maps.append({
            "xt": xts[b],
            "wq": _tile_w(Wq),
            "wk": _tile_w(Wk),
            "bq": np.ascontiguousarray(bq[cols]),
            "bk": np.ascontiguousarray(bk[cols]),
            "maskb": maskbs[b],
        })
    return in_maps


def _gather(results):
    full = np.empty((B, S, D), dtype=np.float32)
    for c in range(NCORES):
        b = c // (NCORES // B)
        hg = c % (NCORES // B)
        cols = slice(hg * 2 * 128, (hg + 1) * 2 * 128)
        r = results[c]["out"].reshape(NG, S, 128)
        full[b, :, cols] = np.concatenate([r[0], r[1]], axis=1)
    return full


def run_sharded(in_maps, **kw):
    from concourse.bass_utils import run_bass_kernel_spmd
    nc = _get_nc()
    return run_bass_kernel_spmd(nc, in_maps, list(range(NCORES)), **kw)


def kernel(hidden_states, attention_mask, Wq, bq, Wk, bk):
    in_maps = _shard_inputs(hidden_states, attention_mask, Wq, bq, Wk, bk)
    res = run_sharded(in_maps)
    return _gather(res.results)
